# revision 40
# baseline (speedup 1.0000x reference)
"""Causal multi-head attention block (qkv proj + causal softmax attention + o proj)
for Trainium2, sharded over 8 NeuronCores: data-parallel on batch (B=2),
tensor-parallel on heads (4 heads/core) with an on-device ReduceScatter after
the o-projection partial products.

v2 layout/schedule (per core, its batch b and 4 heads):
  - host pre-packs every tensor into its SBUF-resident [128, N] layout so each
    load is ONE large DMA (x: 4x1MB, wqk: 2x512KB, ...); loads are issued on
    the Sync queue, x/ow/ob on the Scalar queue, stores + collective triggers
    on the GpSimd queue so no queue blocks another's critical path
  - projections are pipelined INTO the attention weave: chunk c+1's qk/v
    projection matmul groups fill TensorE while block c's attention is
    ScalarE(exp)-bound, keeping the PE dense so HAM stays un-throttled
  - per (qblock, head-pair, kchunk): the two heads' S^T matmuls (K=64,
    disjoint PE row groups, concurrent) write adjacent PSUM banks of one
    [128,1024] tile; off-diagonal chunks take ONE combined [128,1024] exp on
    ScalarE, diagonal chunks take two [128,512-off] exps
  - causal mask via gpsimd affine_select narrowed to the 128 diagonal columns
  - V carries a ones column per head (M=65 PV matmuls) so the PV accumulation
    also yields the softmax denominator; normalization = reciprocal +
    partition-broadcast matmul + multiply as before
  - o-projection groups + per-chunk ReduceScatter weave into the next block's
    stream; RS results land in internal dram tiles whose copies to the kernel
    outputs are deferred to the drain (the Sync queue carries only the initial
    loads, so a copy blocking on a collective never stalls compute)
"""

import numpy as np
import ml_dtypes

import sys
for _p in ("/opt/trn_rl_repo", "/root/.axon_site/_ro/trn_rl_repo"):
    if _p not in sys.path:
        sys.path.append(_p)

B = 2
T = 2048
E = 1024
H = 16
HD = 64
NCORES = 8
TP = 4              # tensor-parallel degree (cores per batch)
HPC = H // TP       # heads per core = 4
FPC = HPC * HD      # q/k/v feature dims per core = 256
VA = HPC * (HD + 1)  # v features with ones column = 260
QB = 512            # q block size (matmul moving free dim)
KC = 128            # k chunk (contraction tile for PV)
TCH = 512           # token chunk for projections

CHUNK_DEFS = [(0, 512), (512, 512), (1024, 512), (1536, 256), (1792, 256)]

_CACHE = {}


def _build_program(t=T):
    import concourse.bass as bass
    import concourse.bacc as bacc_mod
    import concourse.tile as tile
    import concourse.mybir as mybir

    dt = mybir.dt
    f32 = dt.float32
    bf16 = dt.bfloat16
    AF = mybir.ActivationFunctionType

    nt = t // 128          # token tiles = 16
    ntc = t // TCH         # token chunks for projections = 4
    nqb = t // QB          # q blocks = 4
    NKC = E // 128         # contraction chunks for projections = 8

    nc = bacc_mod.Bacc(None, num_devices=NCORES)

    # host-packed parameters (already in SBUF partition-major layout)
    xP = nc.declare_dram_parameter("xP", [128, ntc * NKC * TCH], bf16, isOutput=False)
    wqkP = nc.declare_dram_parameter("wqkP", [128, NKC * 512], bf16, isOutput=False)
    bqk4 = nc.declare_dram_parameter("bqk4", [128, 4], f32, isOutput=False)
    wvP = nc.declare_dram_parameter("wvP", [128, NKC * VA], bf16, isOutput=False)
    bvaP = nc.declare_dram_parameter("bvaP", [128, VA], f32, isOutput=False)
    owP = nc.declare_dram_parameter("owP", [128, 2 * E], bf16, isOutput=False)
    obP = nc.declare_dram_parameter("obP", [128, E], f32, isOutput=False)
    ones64 = nc.declare_dram_parameter("ones64", [1, 64], bf16, isOutput=False)
    rs_outs = [
        nc.declare_dram_parameter(f"rs{k}", [nr // TP, E], bf16, isOutput=True)
        for k, (_, nr) in enumerate(CHUNK_DEFS)
    ]

    with tile.TileContext(nc) as tc, nc.allow_low_precision(
        reason="float32r is 4-byte; fp32 bits"
    ):
        with (
            tc.tile_pool(name="consts", bufs=1) as consts,
            tc.tile_pool(name="resident", bufs=1) as res,
            tc.tile_pool(name="dram", bufs=1, space="DRAM") as dram,
        ):
            # ---- resident SBUF tensors -----------------------------------------
            xs = [
                consts.tile([128, NKC * TCH], bf16, name=f"xs{c}", tag=f"xs{c}")
                for c in range(ntc)
            ]
            wqk_sb = consts.tile([128, NKC * 512], bf16, name="wqk", tag="wqk")
            wv_sb = consts.tile([128, NKC * VA], bf16, name="wv", tag="wv")
            ow_sb = consts.tile([128, 2 * E], bf16, name="ow", tag="ow")
            bqk_sb = consts.tile([128, 4], f32, name="bqk", tag="bqk")
            bva_sb = consts.tile([128, VA], f32, name="bva", tag="bva")
            ob_sb = consts.tile([128, E], f32, name="ob", tag="ob")
            ones1 = consts.tile([1, 64], bf16, name="ones1", tag="ones1")

            # qks[0..1][tcix]: q^T features (head pair j -> rows (h%2)*64)
            # qks[2..3][tcix]: k^T features
            qks = [
                [
                    res.tile([128, TCH], bf16, name=f"qk{i}_{tx}", tag=f"qk{i}_{tx}")
                    for tx in range(ntc)
                ]
                for i in range(4)
            ]
            va_sbs = [
                res.tile([128, VA], bf16, name=f"va{i}", tag=f"va{i}") for i in range(nt)
            ]
            y_sbs = [
                [
                    res.tile([128, TCH], bf16, name=f"y{i}_{tx}", tag=f"y{i}_{tx}")
                    for tx in range(ntc)
                ]
                for i in range(FPC // 128)
            ]

            o_chunks = [
                dram.tile([nr, E], bf16, name=f"oc{ch}", tag=f"oc{ch}")
                for ch, (_, nr) in enumerate(CHUNK_DEFS)
            ]

            # ---- input loads ---------------------------------------------------
            # sync queue: weights/biases; scalar queue: x chunks + o-proj consts
            nc.sync.dma_start(wqk_sb[:, 0:2048], wqkP[:, 0:2048])
            nc.scalar.dma_start(xs[0][:, 0:2048], xP[:, 0:2048])
            nc.scalar.dma_start(xs[0][:, 2048:4096], xP[:, 2048:4096])
            nc.sync.dma_start(wqk_sb[:, 2048:4096], wqkP[:, 2048:4096])
            nc.sync.dma_start(wv_sb[:], wvP[:])
            nc.sync.dma_start(bqk_sb[:], bqk4[:])
            nc.sync.dma_start(bva_sb[:], bvaP[:])
            nc.sync.dma_start(ones1[:], ones64[:])
            for c in range(1, ntc):
                nc.scalar.dma_start(
                    xs[c][:], xP[:, c * NKC * TCH:(c + 1) * NKC * TCH]
                )
            nc.scalar.dma_start(ow_sb[:], owP[:])
            nc.scalar.dma_start(ob_sb[:], obP[:])

            with (
                tc.tile_pool(name="pt", bufs=4) as pt_pool,
                tc.tile_pool(name="rrow", bufs=2) as rrow_pool,
                tc.tile_pool(name="rbs", bufs=2) as rbs_pool,
                tc.tile_pool(name="osb", bufs=4) as osb_pool,
                tc.tile_pool(name="pst", bufs=2, space="PSUM") as pst_pool,
                tc.tile_pool(name="py", bufs=2, space="PSUM") as py_pool,
                tc.tile_pool(name="pp", bufs=2, space="PSUM") as pp_pool,
            ):
                # ---- projection groups (weavable units) ------------------------
                def proj_qk_group(tcix, ft):
                    ps = pp_pool.tile([128, TCH], f32, name="pp", tag="pp")
                    for kc in range(NKC):
                        nc.tensor.matmul(
                            ps[:],
                            lhsT=wqk_sb[:, kc * 512 + ft * 128:kc * 512 + (ft + 1) * 128],
                            rhs=xs[tcix][:, kc * TCH:(kc + 1) * TCH],
                            start=(kc == 0),
                            stop=(kc == NKC - 1),
                        )
                    nc.vector.tensor_scalar_add(
                        qks[ft][tcix][:], ps[:], bqk_sb[:, ft:ft + 1]
                    )

                def proj_v_group(tcix, ts):
                    ps = pp_pool.tile([128, VA], f32, name="ppv", tag="pp")
                    ti = tcix * (TCH // 128) + ts
                    for kc in range(NKC):
                        nc.tensor.matmul(
                            ps[:],
                            lhsT=xs[tcix][:, kc * TCH + ts * 128:kc * TCH + (ts + 1) * 128],
                            rhs=wv_sb[:, kc * VA:(kc + 1) * VA],
                            start=(kc == 0),
                            stop=(kc == NKC - 1),
                        )
                    nc.vector.tensor_add(va_sbs[ti][:], ps[:], bva_sb[:])

                # ---- o-projection + ReduceScatter ------------------------------
                def emit_o_tile(tt, alt=False):
                    row = tt * 128
                    ch = next(
                        i for i, (st_, nr) in enumerate(CHUNK_DEFS)
                        if st_ <= row < st_ + nr
                    )
                    half = (row - CHUNK_DEFS[ch][0]) // 128
                    col = (tt % (TCH // 128)) * 128
                    osb = osb_pool.tile([128, E], bf16, name="osb", tag="osb")
                    for ec in range(E // 512):
                        if alt and ec == 1:
                            psum = py_pool.tile([128, 512], f32, name="poa",
                                                tag="ypsum")
                        else:
                            psum = pp_pool.tile([128, 512], f32, name="po",
                                                tag="pp")
                        for dc in range(FPC // 128):
                            nc.tensor.matmul(
                                psum[:],
                                lhsT=y_sbs[dc][tt // (TCH // 128)][:, col:col + 128],
                                rhs=ow_sb[:, dc * E + ec * 512:
                                          dc * E + (ec + 1) * 512],
                                start=(dc == 0),
                                stop=(dc == FPC // 128 - 1),
                            )
                        nc.vector.tensor_add(
                            osb[:, ec * 512:(ec + 1) * 512], psum[:],
                            ob_sb[:, ec * 512:(ec + 1) * 512]
                        )
                    nc.gpsimd.dma_start(
                        o_chunks[ch][half * 128:(half + 1) * 128, :],
                        osb[:],
                    )

                rs_ready = []

                def emit_rs(ch):
                    rs_o = dram.tile(
                        [CHUNK_DEFS[ch][1] // TP, E], bf16,
                        name=f"rso{ch}", tag=f"rso{ch}"
                    )
                    nc.gpsimd.collective_compute(
                        "ReduceScatter",
                        mybir.AluOpType.add,
                        replica_groups=[[0, 1, 2, 3], [4, 5, 6, 7]],
                        ins=[o_chunks[ch].opt()],
                        outs=[rs_o.opt()],
                    )
                    rs_ready.append((ch, rs_o))

                pending = []      # weavable filler closures
                pending_norm = []

                def weave(allow_o=True):
                    # o-groups (and their RS triggers) first so collectives
                    # start as early as possible; fall back to proj fillers
                    # when o-items are still gated on pending norms
                    for idx, item in enumerate(pending):
                        if getattr(item, "needs_y", False):
                            if allow_o:
                                pending.pop(idx)()
                                return
                            continue
                        # never reorder proj ahead of an RS trigger's o-groups
                        pending.pop(idx)()
                        return

                def flush_norm():
                    while pending_norm:
                        pending_norm.pop(0)()

                def queue_proj_q(tcix):
                    # q features (ft 0,1): needed at block tcix's first s_pair
                    for ft in range(2):
                        def _pq(tcix=tcix, ft=ft):
                            proj_qk_group(tcix, ft)
                        _pq.tcix = tcix
                        _pq.kind = "q"
                        pending.append(_pq)

                def queue_proj_kv(tcix):
                    # k features (ft 2,3) + v: first consumed at kc == 4*tcix,
                    # so they can weave deep into block tcix itself
                    for ft in range(2, 4):
                        def _pk(tcix=tcix, ft=ft):
                            proj_qk_group(tcix, ft)
                        _pk.tcix = tcix
                        _pk.kind = "kv"
                        pending.append(_pk)
                    for ts in range(4):
                        def _pv(tcix=tcix, ts=ts):
                            proj_v_group(tcix, ts)
                        _pv.tcix = tcix
                        _pv.kind = "kv"
                        pending.append(_pv)

                def queue_o(qb):
                    for ttl in range(QB // 128):
                        tt = qb * (QB // 128) + ttl
                        def _og(tt=tt):
                            emit_o_tile(tt)
                        _og.is_o = True
                        _og.needs_y = True
                        pending.append(_og)
                        row_end = (tt + 1) * 128
                        for ch, (st_, nr) in enumerate(CHUNK_DEFS):
                            if st_ + nr == row_end:
                                def _rs(ch=ch):
                                    emit_rs(ch)
                                _rs.needs_y = True
                                pending.append(_rs)

                # ---- attention -------------------------------------------------
                # prologue: project chunk 0 (emitted first so TensorE starts as
                # soon as wqk/xs[0] land)
                for ft in range(4):
                    proj_qk_group(0, ft)
                for ts in range(4):
                    proj_v_group(0, ts)
                queue_proj_q(1)
                queue_proj_kv(1)

                for qb in range(nqb):
                    # q tiles (and any stale fillers) must be traced before
                    # this block's first s_pair reads them
                    for item in [p for p in pending
                                 if getattr(p, "tcix", 99) < qb
                                 or (getattr(p, "tcix", 99) == qb
                                     and getattr(p, "kind", "") == "q")]:
                        pending.remove(item)
                        item()
                    niter = (qb + 1) * 4 * 2  # chunk iters this block
                    for j in range(2):
                        y_ps = [
                            py_pool.tile([HD + 1, QB], f32, name="ypsum", tag="ypsum")
                            for _ in range(2)
                        ]
                        nkc = (qb + 1) * (QB // KC)

                        def s_pair(kc, qb=qb, j=j):
                            dj = kc - qb * (QB // KC)
                            off = max(0, dj) * KC
                            st = pst_pool.tile([KC, 2 * QB], f32, name="st", tag="st")
                            for hh in range(2):
                                prow = hh * 64
                                ksl = qks[2 + j][kc // (TCH // KC)][
                                    prow:prow + 64,
                                    (kc % (TCH // KC)) * KC:
                                    (kc % (TCH // KC) + 1) * KC,
                                ]
                                qsl = qks[j][qb][prow:prow + 64, off:]
                                nc.tensor.matmul(
                                    st[:, hh * QB + off:(hh + 1) * QB],
                                    lhsT=ksl, rhs=qsl,
                                    start=True, stop=True,
                                )
                            pt = pt_pool.tile([KC, 2 * QB], bf16, name="pt", tag="pt")
                            if dj < 0:
                                # off-diagonal: one combined exp over both heads
                                nc.scalar.activation(pt[:], st[:], AF.Exp)
                            else:
                                # one exp over both heads' valid regions via a
                                # strided [p, 2, QB-off] view
                                st3 = st[:].rearrange("p (h q) -> p h q", h=2)
                                pt3 = pt[:].rearrange("p (h q) -> p h q", h=2)
                                nc.scalar.activation(
                                    pt3[:, :, off:], st3[:, :, off:], AF.Exp
                                )
                                for hh in range(2):
                                    # causal mask: only the 128 diagonal cols
                                    nc.gpsimd.affine_select(
                                        out=pt[:, hh * QB + off:hh * QB + off + KC],
                                        in_=pt[:, hh * QB + off:hh * QB + off + KC],
                                        compare_op=mybir.AluOpType.is_ge,
                                        fill=0.0,
                                        base=0,
                                        channel_multiplier=-1,
                                        pattern=[[1, KC]],
                                    )
                            return pt

                        def pv_pair(kc, pt, y_ps=y_ps, j=j, nkc=nkc, qb=qb):
                            off = max(0, kc - qb * (QB // KC)) * KC
                            for hh in range(2):
                                h = 2 * j + hh
                                nc.tensor.matmul(
                                    y_ps[hh][:, off:],
                                    lhsT=va_sbs[kc][:, h * (HD + 1):
                                                    (h + 1) * (HD + 1)],
                                    rhs=pt[:, hh * QB + off:(hh + 1) * QB],
                                    start=(kc == 0),
                                    stop=(kc == nkc - 1),
                                )

                        # lag-1 pipeline with one weave slot per chunk
                        pend = {}
                        for kc in range(nkc):
                            if j == 0 and kc == qb * 4 and qb > 0:
                                # this block's own k/v tiles are read from the
                                # diagonal group on: force-trace stragglers
                                for item in [p for p in pending
                                             if getattr(p, "tcix", 99) <= qb]:
                                    pending.remove(item)
                                    item()
                            pend[kc] = s_pair(kc)
                            if kc == 2:
                                flush_norm()
                            if len(pending) * 2 >= niter or kc % 2 == 1:
                                weave(allow_o=(j == 1 or kc >= 3))
                            if kc >= 1:
                                pv_pair(kc - 1, pend.pop(kc - 1))
                        pv_pair(nkc - 1, pend.pop(nkc - 1))

                        def norm_pair(qb=qb, j=j, y_ps=y_ps):
                            for hh in range(2):
                                prow = hh * 64
                                l_row = rrow_pool.tile(
                                    [1, QB], bf16, name="lrow", tag="lrow"
                                )
                                nc.vector.tensor_copy(
                                    l_row[:], y_ps[hh][HD:HD + 1, :]
                                )
                                rb = pst_pool.tile(
                                    [64, QB], f32, name="rb", tag="st"
                                )
                                nc.tensor.matmul(
                                    rb[:], lhsT=ones1[:], rhs=l_row[:],
                                    start=True, stop=True,
                                )
                                rbs = rbs_pool.tile(
                                    [64, QB], f32, name="rbs", tag="rbs"
                                )
                                nc.vector.reciprocal_approx_fast(rbs[:], rb[:])
                                nc.vector.tensor_mul(
                                    y_sbs[j][qb][prow:prow + 64, :],
                                    y_ps[hh][0:HD, :],
                                    rbs[:],
                                )

                        pending_norm.append(norm_pair)

                    if qb + 1 < nqb:
                        queue_o(qb)
                        if qb + 2 < ntc:
                            # q part weaves during qb+1 (needed at qb+2 start);
                            # kv part is queued a block later so it weaves
                            # inside qb+2 itself, filling its ScalarE-bound tail
                            queue_proj_q(qb + 2)
                        if 1 <= qb and qb + 1 < ntc:
                            queue_proj_kv(qb + 1)

                # drain: final norms, then remaining o groups alternating psum
                # slots (ypsum slots are free once norms are emitted)
                flush_norm()
                queue_o(nqb - 1)
                while pending:
                    item = pending.pop(0)
                    if getattr(item, "is_o", False):
                        (tt,) = item.__defaults__
                        emit_o_tile(tt, alt=True)
                    else:
                        item()
                # output copies: by now the early chunks' collectives are done,
                # so these waits are free; only the final chunks' copies wait
                for ch, rs_o in rs_ready:
                    nc.sync.dma_start(rs_outs[ch][:], rs_o[:])

    nc.finalize()
    return nc


def _shard_inputs(x, qkv_w, qkv_b, o_w, o_b, t=T):
    """Build the 8 per-core input maps (pre-packed into SBUF layouts)."""
    scale = 1.0 / np.sqrt(HD)
    NKC = E // 128
    ntc = t // TCH
    ob_full = np.ascontiguousarray(
        np.broadcast_to((o_b / TP).reshape(1, E).astype(np.float32), (128, E))
    )
    in_maps = []
    for c in range(NCORES):
        b, tp = c // TP, c % TP
        qr = slice(FPC * tp, FPC * (tp + 1))
        kr = slice(E + FPC * tp, E + FPC * (tp + 1))
        vr = slice(2 * E + FPC * tp, 2 * E + FPC * (tp + 1))

        # x packed: [128, tcix, kc, 512] with x[p, c, kc, f] = x[b, c*512+f, kc*128+p]
        xb = np.asarray(x[b, :t, :], np.float32)  # [T, E]
        xr = xb.reshape(ntc, TCH, NKC, 128)       # [c, f, kc, p]
        xP = np.ascontiguousarray(
            xr.transpose(3, 0, 2, 1).reshape(128, ntc * NKC * TCH)
        ).astype(ml_dtypes.bfloat16)

        # wqk packed: [128, kc*512 + ft*128 + m] = W^T[kc*128+p, ft*128+m]
        wqkT_c = np.empty((E, 2 * FPC), np.float32)
        wqkT_c[:, :FPC] = qkv_w[qr, :].T * scale
        wqkT_c[:, FPC:] = qkv_w[kr, :].T
        wqkP = np.ascontiguousarray(
            wqkT_c.reshape(NKC, 128, 512).transpose(1, 0, 2).reshape(128, NKC * 512)
        ).astype(ml_dtypes.bfloat16)

        bqk_c = np.concatenate([qkv_b[qr] * scale, qkv_b[kr]]).astype(np.float32)
        bqk4 = np.ascontiguousarray(bqk_c.reshape(4, 128).T)  # [128, 4]

        wvT_c = np.zeros((E, VA), np.float32)
        bva_c = np.zeros((1, VA), np.float32)
        wv = qkv_w[vr, :].T  # [E, 256]
        bv = qkv_b[vr]
        for h in range(HPC):
            wvT_c[:, h * (HD + 1):h * (HD + 1) + HD] = wv[:, h * HD:(h + 1) * HD]
            bva_c[0, h * (HD + 1):h * (HD + 1) + HD] = bv[h * HD:(h + 1) * HD]
            bva_c[0, h * (HD + 1) + HD] = 1.0
        wvP = np.ascontiguousarray(
            wvT_c.reshape(NKC, 128, VA).transpose(1, 0, 2).reshape(128, NKC * VA)
        ).astype(ml_dtypes.bfloat16)
        bva_t = np.ascontiguousarray(np.broadcast_to(bva_c, (128, VA)))

        owT_c = np.asarray(o_w[:, FPC * tp:FPC * (tp + 1)].T, np.float32)  # [256, E]
        owP = np.ascontiguousarray(
            owT_c.reshape(2, 128, E).transpose(1, 0, 2).reshape(128, 2 * E)
        ).astype(ml_dtypes.bfloat16)

        in_maps.append(
            {
                "xP": xP,
                "wqkP": wqkP,
                "bqk4": bqk4,
                "wvP": wvP,
                "bvaP": bva_t,
                "owP": owP,
                "obP": ob_full,
                "ones64": np.ones((1, 64), ml_dtypes.bfloat16),
            }
        )
    return in_maps


def _run(in_maps, t=T, trace=False):
    from concourse import bass_utils

    key = ("prog", t)
    if key not in _CACHE:
        _CACHE[key] = _build_program(t)
    nc = _CACHE[key]
    res = bass_utils.run_bass_kernel_spmd(
        nc, in_maps, list(range(NCORES)), trace=trace
    )
    return res


def kernel(x, qkv_w, qkv_b, o_w, o_b):
    x = np.asarray(x, np.float32)
    qkv_w = np.asarray(qkv_w, np.float32)
    qkv_b = np.asarray(qkv_b, np.float32)
    o_w = np.asarray(o_w, np.float32)
    o_b = np.asarray(o_b, np.float32)

    in_maps = _shard_inputs(x, qkv_w, qkv_b, o_w, o_b)
    res = _run(in_maps)
    return assemble(res.results, T)


def assemble(results, t):
    """RS chunk k covers token rows [start_k, start_k + nr_k); within a chunk,
    group member i holds sub-rows [i*nr/4, (i+1)*nr/4)."""
    out = np.empty((B, t, E), np.float32)
    for c in range(NCORES):
        b, tp = c // TP, c % TP
        for k, (st_, nr) in enumerate(CHUNK_DEFS):
            sh = nr // TP
            out[b, st_ + tp * sh: st_ + (tp + 1) * sh, :] = np.asarray(
                results[c][f"rs{k}"]
            ).astype(np.float32)
    return out


# revision 41
# speedup vs baseline: 1.0982x; 1.0982x over previous
"""Causal multi-head attention block (qkv proj + causal softmax attention + o proj)
for Trainium2, sharded over 8 NeuronCores: data-parallel on batch (B=2),
tensor-parallel on heads (4 heads/core) with an on-device ReduceScatter after
the o-projection partial products.

v2 layout/schedule (per core, its batch b and 4 heads):
  - host pre-packs every tensor into its SBUF-resident [128, N] layout so each
    load is ONE large DMA (x: 4x1MB, wqk: 2x512KB, ...); loads are issued on
    the Sync queue, x/ow/ob on the Scalar queue, stores + collective triggers
    on the GpSimd queue so no queue blocks another's critical path
  - projections are pipelined INTO the attention weave: chunk c+1's qk/v
    projection matmul groups fill TensorE while block c's attention is
    ScalarE(exp)-bound, keeping the PE dense so HAM stays un-throttled
  - per (qblock, head-pair, kchunk): the two heads' S^T matmuls (K=64,
    disjoint PE row groups, concurrent) write adjacent PSUM banks of one
    [128,1024] tile; off-diagonal chunks take ONE combined [128,1024] exp on
    ScalarE, diagonal chunks take two [128,512-off] exps
  - causal mask via gpsimd affine_select narrowed to the 128 diagonal columns
  - V carries a ones column per head (M=65 PV matmuls) so the PV accumulation
    also yields the softmax denominator; normalization = reciprocal +
    partition-broadcast matmul + multiply as before
  - o-projection groups + per-chunk ReduceScatter weave into the next block's
    stream; RS results land in internal dram tiles whose copies to the kernel
    outputs are deferred to the drain (the Sync queue carries only the initial
    loads, so a copy blocking on a collective never stalls compute)
"""

import numpy as np
import ml_dtypes

import sys
for _p in ("/opt/trn_rl_repo", "/root/.axon_site/_ro/trn_rl_repo"):
    if _p not in sys.path:
        sys.path.append(_p)

B = 2
T = 2048
E = 1024
H = 16
HD = 64
NCORES = 8
TP = 4              # tensor-parallel degree (cores per batch)
HPC = H // TP       # heads per core = 4
FPC = HPC * HD      # q/k/v feature dims per core = 256
VA = HPC * (HD + 1)  # v features with ones column = 260
QB = 512            # q block size (matmul moving free dim)
KC = 128            # k chunk (contraction tile for PV)
TCH = 512           # token chunk for projections

CHUNK_DEFS = [(0, 512), (512, 512), (1024, 512), (1536, 256), (1792, 256)]

_CACHE = {}


def _build_program(t=T):
    import concourse.bass as bass
    import concourse.bacc as bacc_mod
    import concourse.tile as tile
    import concourse.mybir as mybir

    dt = mybir.dt
    f32 = dt.float32
    bf16 = dt.bfloat16
    AF = mybir.ActivationFunctionType

    nt = t // 128          # token tiles = 16
    ntc = t // TCH         # token chunks for projections = 4
    nqb = t // QB          # q blocks = 4
    NKC = E // 128         # contraction chunks for projections = 8

    nc = bacc_mod.Bacc(None, num_devices=NCORES)

    # host-packed parameters (already in SBUF partition-major layout)
    xP = nc.declare_dram_parameter("xP", [128, ntc * NKC * TCH], bf16, isOutput=False)
    wqkP = nc.declare_dram_parameter("wqkP", [128, NKC * 512], bf16, isOutput=False)
    bqk4 = nc.declare_dram_parameter("bqk4", [128, 4], f32, isOutput=False)
    wvP = nc.declare_dram_parameter("wvP", [128, NKC * VA], bf16, isOutput=False)
    bvaP = nc.declare_dram_parameter("bvaP", [128, VA], f32, isOutput=False)
    owP = nc.declare_dram_parameter("owP", [128, 2 * E], bf16, isOutput=False)
    obP = nc.declare_dram_parameter("obP", [128, E], f32, isOutput=False)
    ones64 = nc.declare_dram_parameter("ones64", [1, 64], bf16, isOutput=False)
    rs_outs = [
        nc.declare_dram_parameter(f"rs{k}", [nr // TP, E], bf16, isOutput=True)
        for k, (_, nr) in enumerate(CHUNK_DEFS)
    ]

    with tile.TileContext(nc) as tc, nc.allow_low_precision(
        reason="float32r is 4-byte; fp32 bits"
    ):
        with (
            tc.tile_pool(name="consts", bufs=1) as consts,
            tc.tile_pool(name="resident", bufs=1) as res,
            tc.tile_pool(name="dram", bufs=1, space="DRAM") as dram,
        ):
            # ---- resident SBUF tensors -----------------------------------------
            xs = [
                consts.tile([128, NKC * TCH], bf16, name=f"xs{c}", tag=f"xs{c}")
                for c in range(ntc)
            ]
            wqk_sb = consts.tile([128, NKC * 512], bf16, name="wqk", tag="wqk")
            wv_sb = consts.tile([128, NKC * VA], bf16, name="wv", tag="wv")
            ow_sb = consts.tile([128, 2 * E], bf16, name="ow", tag="ow")
            bqk_sb = consts.tile([128, 4], f32, name="bqk", tag="bqk")
            bva_sb = consts.tile([128, VA], f32, name="bva", tag="bva")
            ob_sb = consts.tile([128, E], f32, name="ob", tag="ob")
            ones1 = consts.tile([1, 64], bf16, name="ones1", tag="ones1")

            # qks[0..1][tcix]: q^T features (head pair j -> rows (h%2)*64)
            # qks[2..3][tcix]: k^T features
            qks = [
                [
                    res.tile([128, TCH], bf16, name=f"qk{i}_{tx}", tag=f"qk{i}_{tx}")
                    for tx in range(ntc)
                ]
                for i in range(4)
            ]
            va_sbs = [
                res.tile([128, VA], bf16, name=f"va{i}", tag=f"va{i}") for i in range(nt)
            ]
            y_sbs = [
                [
                    res.tile([128, TCH], bf16, name=f"y{i}_{tx}", tag=f"y{i}_{tx}")
                    for tx in range(ntc)
                ]
                for i in range(FPC // 128)
            ]

            o_chunks = [
                dram.tile([nr, E], bf16, name=f"oc{ch}", tag=f"oc{ch}")
                for ch, (_, nr) in enumerate(CHUNK_DEFS)
            ]

            # ---- input loads ---------------------------------------------------
            # sync queue: weights/biases; scalar queue: x chunks + o-proj consts
            # first 512-col slices land first so the first proj matmul (which
            # only needs kc=0) can start ~4us earlier
            nc.sync.dma_start(wqk_sb[:, 0:512], wqkP[:, 0:512])
            nc.scalar.dma_start(xs[0][:, 0:512], xP[:, 0:512])
            nc.sync.dma_start(wqk_sb[:, 512:2048], wqkP[:, 512:2048])
            nc.scalar.dma_start(xs[0][:, 512:2048], xP[:, 512:2048])
            nc.scalar.dma_start(xs[0][:, 2048:4096], xP[:, 2048:4096])
            nc.sync.dma_start(wqk_sb[:, 2048:4096], wqkP[:, 2048:4096])
            nc.sync.dma_start(wv_sb[:], wvP[:])
            nc.sync.dma_start(bqk_sb[:], bqk4[:])
            nc.sync.dma_start(bva_sb[:], bvaP[:])
            nc.sync.dma_start(ones1[:], ones64[:])
            for c in range(1, ntc):
                nc.scalar.dma_start(
                    xs[c][:], xP[:, c * NKC * TCH:(c + 1) * NKC * TCH]
                )
            nc.scalar.dma_start(ow_sb[:], owP[:])
            nc.scalar.dma_start(ob_sb[:], obP[:])

            with (
                tc.tile_pool(name="pt", bufs=4) as pt_pool,
                tc.tile_pool(name="rrow", bufs=2) as rrow_pool,
                tc.tile_pool(name="rbs", bufs=2) as rbs_pool,
                tc.tile_pool(name="osb", bufs=4) as osb_pool,
                tc.tile_pool(name="pst", bufs=2, space="PSUM") as pst_pool,
                tc.tile_pool(name="py", bufs=2, space="PSUM") as py_pool,
                tc.tile_pool(name="pp", bufs=2, space="PSUM") as pp_pool,
            ):
                # ---- projection groups (weavable units) ------------------------
                def proj_qk_group(tcix, ft):
                    ps = pp_pool.tile([128, TCH], f32, name="pp", tag="pp")
                    for kc in range(NKC):
                        nc.tensor.matmul(
                            ps[:],
                            lhsT=wqk_sb[:, kc * 512 + ft * 128:kc * 512 + (ft + 1) * 128],
                            rhs=xs[tcix][:, kc * TCH:(kc + 1) * TCH],
                            start=(kc == 0),
                            stop=(kc == NKC - 1),
                        )
                    nc.vector.tensor_scalar_add(
                        qks[ft][tcix][:], ps[:], bqk_sb[:, ft:ft + 1]
                    )

                def proj_v_group(tcix, ts):
                    ps = pp_pool.tile([128, VA], f32, name="ppv", tag="pp")
                    ti = tcix * (TCH // 128) + ts
                    for kc in range(NKC):
                        nc.tensor.matmul(
                            ps[:],
                            lhsT=xs[tcix][:, kc * TCH + ts * 128:kc * TCH + (ts + 1) * 128],
                            rhs=wv_sb[:, kc * VA:(kc + 1) * VA],
                            start=(kc == 0),
                            stop=(kc == NKC - 1),
                        )
                    nc.vector.tensor_add(va_sbs[ti][:], ps[:], bva_sb[:])

                # ---- o-projection + ReduceScatter ------------------------------
                def emit_o_tile(tt, alt=False):
                    row = tt * 128
                    ch = next(
                        i for i, (st_, nr) in enumerate(CHUNK_DEFS)
                        if st_ <= row < st_ + nr
                    )
                    half = (row - CHUNK_DEFS[ch][0]) // 128
                    col = (tt % (TCH // 128)) * 128
                    osb = osb_pool.tile([128, E], bf16, name="osb", tag="osb")
                    for ec in range(E // 512):
                        if alt and ec == 1:
                            psum = py_pool.tile([128, 512], f32, name="poa",
                                                tag="ypsum")
                        else:
                            psum = pp_pool.tile([128, 512], f32, name="po",
                                                tag="pp")
                        for dc in range(FPC // 128):
                            nc.tensor.matmul(
                                psum[:],
                                lhsT=y_sbs[dc][tt // (TCH // 128)][:, col:col + 128],
                                rhs=ow_sb[:, dc * E + ec * 512:
                                          dc * E + (ec + 1) * 512],
                                start=(dc == 0),
                                stop=(dc == FPC // 128 - 1),
                            )
                        nc.vector.tensor_add(
                            osb[:, ec * 512:(ec + 1) * 512], psum[:],
                            ob_sb[:, ec * 512:(ec + 1) * 512]
                        )
                    nc.gpsimd.dma_start(
                        o_chunks[ch][half * 128:(half + 1) * 128, :],
                        osb[:],
                    )

                rs_ready = []

                def emit_rs(ch):
                    rs_o = dram.tile(
                        [CHUNK_DEFS[ch][1] // TP, E], bf16,
                        name=f"rso{ch}", tag=f"rso{ch}"
                    )
                    nc.gpsimd.collective_compute(
                        "ReduceScatter",
                        mybir.AluOpType.add,
                        replica_groups=[[0, 1, 2, 3], [4, 5, 6, 7]],
                        ins=[o_chunks[ch].opt()],
                        outs=[rs_o.opt()],
                    )
                    rs_ready.append((ch, rs_o))

                pending = []      # weavable filler closures
                pending_norm = []

                def weave(allow_o=True):
                    # o-groups (and their RS triggers) first so collectives
                    # start as early as possible; fall back to proj fillers
                    # when o-items are still gated on pending norms
                    for idx, item in enumerate(pending):
                        if getattr(item, "needs_y", False):
                            if allow_o:
                                pending.pop(idx)()
                                return
                            continue
                        # never reorder proj ahead of an RS trigger's o-groups
                        pending.pop(idx)()
                        return

                def flush_norm():
                    while pending_norm:
                        pending_norm.pop(0)()

                def queue_proj_q(tcix):
                    # q features (ft 0,1): needed at block tcix's first s_pair
                    for ft in range(2):
                        def _pq(tcix=tcix, ft=ft):
                            proj_qk_group(tcix, ft)
                        _pq.tcix = tcix
                        _pq.kind = "q"
                        pending.append(_pq)

                def queue_proj_kv(tcix):
                    # k features (ft 2,3) + v: first consumed at kc == 4*tcix,
                    # so they can weave deep into block tcix itself
                    for ft in range(2, 4):
                        def _pk(tcix=tcix, ft=ft):
                            proj_qk_group(tcix, ft)
                        _pk.tcix = tcix
                        _pk.kind = "kv"
                        pending.append(_pk)
                    for ts in range(4):
                        def _pv(tcix=tcix, ts=ts):
                            proj_v_group(tcix, ts)
                        _pv.tcix = tcix
                        _pv.kind = "kv"
                        pending.append(_pv)

                def queue_o(qb):
                    for ttl in range(QB // 128):
                        tt = qb * (QB // 128) + ttl
                        def _og(tt=tt):
                            emit_o_tile(tt)
                        _og.is_o = True
                        _og.needs_y = True
                        pending.append(_og)
                        row_end = (tt + 1) * 128
                        for ch, (st_, nr) in enumerate(CHUNK_DEFS):
                            if st_ + nr == row_end:
                                def _rs(ch=ch):
                                    emit_rs(ch)
                                _rs.needs_y = True
                                pending.append(_rs)

                # ---- attention -------------------------------------------------
                # prologue: project chunk 0 (emitted first so TensorE starts as
                # soon as wqk/xs[0] land)
                for ft in range(4):
                    proj_qk_group(0, ft)
                for ts in range(4):
                    proj_v_group(0, ts)
                queue_proj_q(1)
                queue_proj_kv(1)

                for qb in range(nqb):
                    # q tiles (and any stale fillers) must be traced before
                    # this block's first s_pair reads them
                    for item in [p for p in pending
                                 if getattr(p, "tcix", 99) < qb
                                 or (getattr(p, "tcix", 99) == qb
                                     and getattr(p, "kind", "") == "q")]:
                        pending.remove(item)
                        item()
                    niter = (qb + 1) * 4 * 2  # chunk iters this block
                    for j in range(2):
                        y_ps = [
                            py_pool.tile([HD + 1, QB], f32, name="ypsum", tag="ypsum")
                            for _ in range(2)
                        ]
                        nkc = (qb + 1) * (QB // KC)

                        def s_pair(kc, qb=qb, j=j):
                            dj = kc - qb * (QB // KC)
                            off = max(0, dj) * KC
                            st = pst_pool.tile([KC, 2 * QB], f32, name="st", tag="st")
                            for hh in range(2):
                                prow = hh * 64
                                ksl = qks[2 + j][kc // (TCH // KC)][
                                    prow:prow + 64,
                                    (kc % (TCH // KC)) * KC:
                                    (kc % (TCH // KC) + 1) * KC,
                                ]
                                qsl = qks[j][qb][prow:prow + 64, off:]
                                nc.tensor.matmul(
                                    st[:, hh * QB + off:(hh + 1) * QB],
                                    lhsT=ksl, rhs=qsl,
                                    start=True, stop=True,
                                )
                            pt = pt_pool.tile([KC, 2 * QB], bf16, name="pt", tag="pt")
                            if dj < 0:
                                # off-diagonal: one combined exp over both heads
                                nc.scalar.activation(pt[:], st[:], AF.Exp)
                            else:
                                # one exp over both heads' valid regions via a
                                # strided [p, 2, QB-off] view
                                st3 = st[:].rearrange("p (h q) -> p h q", h=2)
                                pt3 = pt[:].rearrange("p (h q) -> p h q", h=2)
                                nc.scalar.activation(
                                    pt3[:, :, off:], st3[:, :, off:], AF.Exp
                                )
                                for hh in range(2):
                                    # causal mask: only the 128 diagonal cols
                                    nc.gpsimd.affine_select(
                                        out=pt[:, hh * QB + off:hh * QB + off + KC],
                                        in_=pt[:, hh * QB + off:hh * QB + off + KC],
                                        compare_op=mybir.AluOpType.is_ge,
                                        fill=0.0,
                                        base=0,
                                        channel_multiplier=-1,
                                        pattern=[[1, KC]],
                                    )
                            return pt

                        def pv_pair(kc, pt, y_ps=y_ps, j=j, nkc=nkc, qb=qb):
                            off = max(0, kc - qb * (QB // KC)) * KC
                            for hh in range(2):
                                h = 2 * j + hh
                                nc.tensor.matmul(
                                    y_ps[hh][:, off:],
                                    lhsT=va_sbs[kc][:, h * (HD + 1):
                                                    (h + 1) * (HD + 1)],
                                    rhs=pt[:, hh * QB + off:(hh + 1) * QB],
                                    start=(kc == 0),
                                    stop=(kc == nkc - 1),
                                )

                        # lag-1 pipeline with one weave slot per chunk
                        pend = {}
                        for kc in range(nkc):
                            if j == 0 and kc == qb * 4 and qb > 0:
                                # this block's own k/v tiles are read from the
                                # diagonal group on: force-trace stragglers
                                for item in [p for p in pending
                                             if getattr(p, "tcix", 99) <= qb]:
                                    pending.remove(item)
                                    item()
                            pend[kc] = s_pair(kc)
                            if kc == 2:
                                flush_norm()
                            if len(pending) * 2 >= niter or kc % 2 == 1:
                                weave(allow_o=(j == 1 or kc >= 3))
                            if kc >= 1:
                                pv_pair(kc - 1, pend.pop(kc - 1))
                        pv_pair(nkc - 1, pend.pop(nkc - 1))

                        def norm_pair(qb=qb, j=j, y_ps=y_ps):
                            for hh in range(2):
                                prow = hh * 64
                                l_row = rrow_pool.tile(
                                    [1, QB], bf16, name="lrow", tag="lrow"
                                )
                                nc.vector.tensor_copy(
                                    l_row[:], y_ps[hh][HD:HD + 1, :]
                                )
                                rb = pst_pool.tile(
                                    [64, QB], f32, name="rb", tag="st"
                                )
                                nc.tensor.matmul(
                                    rb[:], lhsT=ones1[:], rhs=l_row[:],
                                    start=True, stop=True,
                                )
                                rbs = rbs_pool.tile(
                                    [64, QB], f32, name="rbs", tag="rbs"
                                )
                                nc.vector.reciprocal_approx_fast(rbs[:], rb[:])
                                nc.vector.tensor_mul(
                                    y_sbs[j][qb][prow:prow + 64, :],
                                    y_ps[hh][0:HD, :],
                                    rbs[:],
                                )

                        pending_norm.append(norm_pair)

                    if qb + 1 < nqb:
                        queue_o(qb)
                        if qb + 2 < ntc:
                            # q part weaves during qb+1 (needed at qb+2 start);
                            # kv part is queued a block later so it weaves
                            # inside qb+2 itself, filling its ScalarE-bound tail
                            queue_proj_q(qb + 2)
                        if 1 <= qb and qb + 1 < ntc:
                            queue_proj_kv(qb + 1)

                # drain: final norms, then remaining o groups alternating psum
                # slots (ypsum slots are free once norms are emitted)
                flush_norm()
                queue_o(nqb - 1)
                while pending:
                    item = pending.pop(0)
                    if getattr(item, "is_o", False):
                        (tt,) = item.__defaults__
                        emit_o_tile(tt, alt=True)
                    else:
                        item()
                # output copies: by now the early chunks' collectives are done,
                # so these waits are free; only the final chunks' copies wait
                for ch, rs_o in rs_ready:
                    nc.sync.dma_start(rs_outs[ch][:], rs_o[:])

    nc.finalize()
    return nc


def _shard_inputs(x, qkv_w, qkv_b, o_w, o_b, t=T):
    """Build the 8 per-core input maps (pre-packed into SBUF layouts)."""
    scale = 1.0 / np.sqrt(HD)
    NKC = E // 128
    ntc = t // TCH
    ob_full = np.ascontiguousarray(
        np.broadcast_to((o_b / TP).reshape(1, E).astype(np.float32), (128, E))
    )
    in_maps = []
    for c in range(NCORES):
        b, tp = c // TP, c % TP
        qr = slice(FPC * tp, FPC * (tp + 1))
        kr = slice(E + FPC * tp, E + FPC * (tp + 1))
        vr = slice(2 * E + FPC * tp, 2 * E + FPC * (tp + 1))

        # x packed: [128, tcix, kc, 512] with x[p, c, kc, f] = x[b, c*512+f, kc*128+p]
        xb = np.asarray(x[b, :t, :], np.float32)  # [T, E]
        xr = xb.reshape(ntc, TCH, NKC, 128)       # [c, f, kc, p]
        xP = np.ascontiguousarray(
            xr.transpose(3, 0, 2, 1).reshape(128, ntc * NKC * TCH)
        ).astype(ml_dtypes.bfloat16)

        # wqk packed: [128, kc*512 + ft*128 + m] = W^T[kc*128+p, ft*128+m]
        wqkT_c = np.empty((E, 2 * FPC), np.float32)
        wqkT_c[:, :FPC] = qkv_w[qr, :].T * scale
        wqkT_c[:, FPC:] = qkv_w[kr, :].T
        wqkP = np.ascontiguousarray(
            wqkT_c.reshape(NKC, 128, 512).transpose(1, 0, 2).reshape(128, NKC * 512)
        ).astype(ml_dtypes.bfloat16)

        bqk_c = np.concatenate([qkv_b[qr] * scale, qkv_b[kr]]).astype(np.float32)
        bqk4 = np.ascontiguousarray(bqk_c.reshape(4, 128).T)  # [128, 4]

        wvT_c = np.zeros((E, VA), np.float32)
        bva_c = np.zeros((1, VA), np.float32)
        wv = qkv_w[vr, :].T  # [E, 256]
        bv = qkv_b[vr]
        for h in range(HPC):
            wvT_c[:, h * (HD + 1):h * (HD + 1) + HD] = wv[:, h * HD:(h + 1) * HD]
            bva_c[0, h * (HD + 1):h * (HD + 1) + HD] = bv[h * HD:(h + 1) * HD]
            bva_c[0, h * (HD + 1) + HD] = 1.0
        wvP = np.ascontiguousarray(
            wvT_c.reshape(NKC, 128, VA).transpose(1, 0, 2).reshape(128, NKC * VA)
        ).astype(ml_dtypes.bfloat16)
        bva_t = np.ascontiguousarray(np.broadcast_to(bva_c, (128, VA)))

        owT_c = np.asarray(o_w[:, FPC * tp:FPC * (tp + 1)].T, np.float32)  # [256, E]
        owP = np.ascontiguousarray(
            owT_c.reshape(2, 128, E).transpose(1, 0, 2).reshape(128, 2 * E)
        ).astype(ml_dtypes.bfloat16)

        in_maps.append(
            {
                "xP": xP,
                "wqkP": wqkP,
                "bqk4": bqk4,
                "wvP": wvP,
                "bvaP": bva_t,
                "owP": owP,
                "obP": ob_full,
                "ones64": np.ones((1, 64), ml_dtypes.bfloat16),
            }
        )
    return in_maps


def _run(in_maps, t=T, trace=False):
    from concourse import bass_utils

    key = ("prog", t)
    if key not in _CACHE:
        _CACHE[key] = _build_program(t)
    nc = _CACHE[key]
    res = bass_utils.run_bass_kernel_spmd(
        nc, in_maps, list(range(NCORES)), trace=trace
    )
    return res


def kernel(x, qkv_w, qkv_b, o_w, o_b):
    x = np.asarray(x, np.float32)
    qkv_w = np.asarray(qkv_w, np.float32)
    qkv_b = np.asarray(qkv_b, np.float32)
    o_w = np.asarray(o_w, np.float32)
    o_b = np.asarray(o_b, np.float32)

    in_maps = _shard_inputs(x, qkv_w, qkv_b, o_w, o_b)
    res = _run(in_maps)
    return assemble(res.results, T)


def assemble(results, t):
    """RS chunk k covers token rows [start_k, start_k + nr_k); within a chunk,
    group member i holds sub-rows [i*nr/4, (i+1)*nr/4)."""
    out = np.empty((B, t, E), np.float32)
    for c in range(NCORES):
        b, tp = c // TP, c % TP
        for k, (st_, nr) in enumerate(CHUNK_DEFS):
            sh = nr // TP
            out[b, st_ + tp * sh: st_ + (tp + 1) * sh, :] = np.asarray(
                results[c][f"rs{k}"]
            ).astype(np.float32)
    return out
